# revision 10
# baseline (speedup 1.0000x reference)
"""Trainium2 Bass kernel for nn_Decoder_9775345565829.

Data-parallel over 8 NeuronCores (B=4096 -> 512/core). Sequential T=20 scan.

Math refactoring (exact algebra):
  z_t = cond_t @ Wce + zbase + zoff[t]     (gate order permuted to [i,f,o,g])
    Wce = We1@We2@Wk[:E]  (rank-5; aug row 5 carries zoff[t])
    zbase = state_h @ (Wk[E:E+H] + Wr)     (step-independent, host-precomputed)
    zoff[t] = cumsum(Wk[E+H:])[t] + bl + (be1@We2+be2)@Wk[:E]
  sigmoid-free gates (single ACT table set "exp_and_others"):
    T* = tanh(z*/2);  2c = (1+Tf)*c0 + (1+Ti)*tanh(zg);  tanh(c) via scale=0.5
    2h = (1+To)*tanh(c);  heads use W_all/2 so p = h @ W_all + b_all
  sampling: host-precomputed threefry gumbel/normal noise; categorical =
  onehot(argmax(logits+gumbel)); sqrt(1-rho^2) by poly (|rho| small).

Device: gates feature-on-partition (8 slices x [128, 512b]); heads/sampling
batch-on-partition via PE transposes.
"""
import os

os.environ.setdefault("JAX_PLATFORMS", "axon,cpu")

import numpy as np

B, T, P, H, E, K, COND = 4096, 20, 20, 256, 128, 5, 5
NCORES, BC, NB = 8, 512, 4
H4 = 4 * H

_CACHE = {}


# ------------------------------------------------------------------ noise ---
def _noise():
    """Replay the reference's jax.random key tree (threefry is platform-
    deterministic). gum [T,B,20] head-major (m,y,f,fa); nrm [T,B,6] cols
    [z1, 0, zy, zf, zfa, z2]."""
    if "noise" in _CACHE:
        return _CACHE["noise"]
    import jax

    try:
        dev = jax.devices("cpu")[0]
    except RuntimeError:
        dev = jax.devices()[0]
    gum = np.zeros((T, B, 20), np.float32)
    nrm = np.zeros((T, B, 6), np.float32)
    with jax.default_device(dev):
        key = jax.random.key(42)
        for t in range(T):
            key, km, ky, kf, kfa = jax.random.split(key, 5)
            k1, k2 = jax.random.split(km)
            gum[t, :, 0:5] = np.asarray(jax.random.gumbel(k1, (B, K)))
            zm = np.asarray(jax.random.normal(k2, (B, 2)))
            nrm[t, :, 0], nrm[t, :, 5] = zm[:, 0], zm[:, 1]
            for j, kh in enumerate((ky, kf, kfa)):
                k1, k2 = jax.random.split(kh)
                gum[t, :, 5 + 5 * j:10 + 5 * j] = np.asarray(
                    jax.random.gumbel(k1, (B, K)))
                nrm[t, :, 2 + j] = np.asarray(
                    jax.random.normal(k2, (B, 1)))[:, 0]
    _CACHE["noise"] = (gum, nrm)
    return gum, nrm


# ---------------------------------------------------------------- weights ---
def _fold(inp):
    f32 = lambda k: np.asarray(inp[k], np.float32)  # noqa: E731
    We1, be1, We2, be2 = f32("We1"), f32("be1"), f32("We2"), f32("be2")
    Wk, Wr, bl = f32("Wk"), f32("Wr"), f32("bl")
    Wm, bm = f32("Wm"), f32("bm")
    Wy, by = f32("Wy"), f32("by")
    Wf, bf = f32("Wf"), f32("bf")
    Wfa, bfa = f32("Wfadj"), f32("bfadj")
    idx = np.arange(H4).reshape(4, H)
    gp = np.concatenate([idx[0], idx[1], idx[3], idx[2]])  # i,f,g,o -> i,f,o,g
    Wk_e, Wk_h, Wk_p = Wk[:E], Wk[E:E + H], Wk[E + H:]
    Wce = (We1 @ We2 @ Wk_e)[:, gp].astype(np.float32)
    Wcomb = (Wk_h + Wr)[:, gp].astype(np.float32)
    embb = be1 @ We2 + be2
    zoff = (np.cumsum(Wk_p, 0)[np.arange(T).clip(0, P - 1)] + bl
            + embb @ Wk_e)[:, gp].astype(np.float32)
    half = np.concatenate([np.full(3 * H, 0.5, np.float32),
                           np.ones(H, np.float32)])
    Wce *= half
    Wcomb *= half
    zoff *= half
    W_all = np.zeros((H, 75), np.float32)
    b_all = np.zeros((75,), np.float32)
    W_all[:, 0:5], b_all[0:5] = Wm[:, 0:5], bm[0:5]
    W_all[:, 5:10], b_all[5:10] = Wy[:, 0:5], by[0:5]
    W_all[:, 10:15], b_all[10:15] = Wf[:, 0:5], bf[0:5]
    W_all[:, 15:20], b_all[15:20] = Wfa[:, 0:5], bfa[0:5]
    W_all[:, 20:45], b_all[20:45] = Wm[:, 5:30], bm[5:30]
    W_all[:, 45:55], b_all[45:55] = Wy[:, 5:15], by[5:15]
    W_all[:, 55:65], b_all[55:65] = Wf[:, 5:15], bf[5:15]
    W_all[:, 65:75], b_all[65:75] = Wfa[:, 5:15], bfa[5:15]
    return Wce, Wcomb, zoff, W_all, b_all


def _slice_id(s, u):  # wave s slot u -> feature slice (order [i,f,o,g])
    return (s, 2 + s, 4 + s, 6 + s)[u]


# ------------------------------------------------------------ bass module ---
def _build():
    if "nc" in _CACHE:
        return _CACHE["nc"]
    import concourse.bass as bass
    import concourse.mybir as mybir
    from concourse import bacc
    from concourse.tile import TileContext
    from concourse.masks import make_identity

    F32, F16, F32R = mybir.dt.float32, mybir.dt.float16, mybir.dt.float32r
    AF, OP = mybir.ActivationFunctionType, mybir.AluOpType
    X = mybir.AxisListType.X

    def rap(src, *dims):
        """new AP reusing src's tensor/partition-dim/offset, with the free
        dims replaced by explicit (step, count) pairs (element units)."""
        return bass.AP(src.tensor, src.offset,
                       [list(src.ap[0])] + [[s_, c_] for (s_, c_) in dims])

    nc = bacc.Bacc(None, target_bir_lowering=False)
    d_sht = nc.dram_tensor("sht", [128, 2, 512], F32R, kind="ExternalInput")
    d_wcb = nc.dram_tensor("wcb", [128, 2, 8, 128], F32R, kind="ExternalInput")
    d_c0t = nc.dram_tensor("c0t", [128, 2, 512], F16, kind="ExternalInput")
    d_wce = nc.dram_tensor("wce", [6, 8 * T, 128], F32R, kind="ExternalInput")
    d_wal = nc.dram_tensor("wal", [128, 2, 75], F16, kind="ExternalInput")
    d_bal = nc.dram_tensor("bal", [75, 1], F32, kind="ExternalInput")
    d_gmb = nc.dram_tensor("gmb", [128, T, 4, 20], F32, kind="ExternalInput")
    d_nrb = nc.dram_tensor("nrb", [128, T, 4, 6], F32, kind="ExternalInput")
    d_ct0 = nc.dram_tensor("ct0", [128, 512], F32R, kind="ExternalInput")
    d_out = nc.dram_tensor("out", [4, 128, T, 75], F32, kind="ExternalOutput")

    with TileContext(nc) as tc:
        with tc.tile_pool(name="const", bufs=1) as cp, \
             tc.tile_pool(name="gw", bufs=2) as gw, \
             tc.tile_pool(name="sm", bufs=3) as sm, \
             tc.tile_pool(name="obp", bufs=3) as obp, \
             tc.tile_pool(name="ctp", bufs=2) as ctp, \
             tc.tile_pool(name="psz", bufs=1, space="PSUM") as psz, \
             tc.tile_pool(name="psp", bufs=1, space="PSUM") as psp, \
             tc.tile_pool(name="psb", bufs=2, space="PSUM") as psb, \
             tc.tile_pool(name="psc", bufs=1, space="PSUM") as psc:

            sht = cp.tile([128, 2, 512], F32R)
            nc.sync.dma_start(out=sht, in_=d_sht[:, :, :])
            wcb = cp.tile([128, 2, 8, 128], F32R)
            nc.sync.dma_start(out=wcb, in_=d_wcb[:, :, :, :])
            c0t = cp.tile([128, 2, 512], F16)
            nc.sync.dma_start(out=c0t, in_=d_c0t[:, :, :])
            wce = cp.tile([128, 8 * T, 128], F32R)
            nc.sync.dma_start(out=wce[0:6, :, :], in_=d_wce[:, :, :])
            wal = cp.tile([128, 2, 75], F16)
            nc.sync.dma_start(out=wal, in_=d_wal[:, :, :])
            bal = cp.tile([128, 1], F32)
            nc.sync.dma_start(out=bal[0:75, :], in_=d_bal[:, :])
            gmb = cp.tile([128, T, 4, 20], F32)
            nc.sync.dma_start(out=gmb, in_=d_gmb[:, :, :, :])
            nrb = cp.tile([128, T, 4, 6], F32)
            nc.sync.dma_start(out=nrb, in_=d_nrb[:, :, :, :])
            ident = cp.tile([128, 128], F32)
            make_identity(nc, ident)

            ct = ctp.tile([128, 512], F32R, tag="ct")
            nc.sync.dma_start(out=ct, in_=d_ct0[:, :])

            for t in range(T):
                h2 = gw.tile([128, 2, 512], F16, tag="h2")

                def flush(pend):
                    # software-pipelined tail of a gate wave: tanh(c) and 2h
                    cs_, th_, s_, bs_ = pend
                    tcn = gw.tile([128, 256], F16, tag="tcn")
                    nc.scalar.activation(tcn, cs_, AF.Tanh, scale=0.5)
                    nc.vector.scalar_tensor_tensor(
                        h2[:, s_, bs_], th_[:, 2, :], 1.0, tcn,
                        OP.add, OP.mult)

                pend = None
                for s in range(2):
                    zt0 = psz.tile([128, 4, 256], F32, tag="zt")
                    zt1 = psz.tile([128, 4, 256], F32, tag="zt")
                    ztb = [zt0, zt1]
                    for u in range(4):
                        sl = _slice_id(s, u)
                        # zbase accumulation first: no dependency on cond,
                        # overlaps the previous step's sampling tail
                        for k2 in range(2):
                            for b in range(2):
                                nc.tensor.matmul(
                                    ztb[b][:, u, :], lhsT=wcb[:, k2, sl, :],
                                    rhs=sht[:, k2, 256 * b:256 * (b + 1)],
                                    start=(k2 == 0), stop=False)
                        for b in range(2):
                            nc.tensor.matmul(
                                ztb[b][:, u, :],
                                lhsT=wce[0:6, 8 * t + 4 * s + u, :],
                                rhs=ct[0:6, 256 * b:256 * (b + 1)],
                                start=False, stop=True)
                    for b in range(2):
                        zt = ztb[b]
                        bs = slice(256 * b, 256 * (b + 1))
                        th = gw.tile([128, 4, 256], F16, tag="th")
                        nc.scalar.activation(th, zt, AF.Tanh)
                        a1 = gw.tile([128, 256], F16, tag="a1")
                        nc.vector.scalar_tensor_tensor(
                            a1, th[:, 1, :], 1.0, c0t[:, s, bs],
                            OP.add, OP.mult)
                        a2 = gw.tile([128, 256], F16, tag="a2")
                        nc.vector.scalar_tensor_tensor(
                            a2, th[:, 0, :], 1.0, th[:, 3, :],
                            OP.add, OP.mult)
                        if pend is not None:
                            flush(pend)
                        cs = gw.tile([128, 256], F16, tag="cs")
                        nc.gpsimd.tensor_add(cs, a1, a2)  # 2c
                        pend = (cs, th, s, bs)
                flush(pend)

                # heads
                pp = psp.tile([128, 512], F32, tag="pp")
                for k2 in range(2):
                    nc.tensor.matmul(pp[0:75, :], lhsT=wal[:, k2, :],
                                     rhs=h2[:, k2, :],
                                     start=(k2 == 0), stop=(k2 == 1))
                pt = sm.tile([128, 512], F32, tag="pt")
                nc.scalar.activation(pt[0:75, :], pp[0:75, :], AF.Identity,
                                     bias=bal[0:75, :])
                pb = psb.tile([128, 4, 75], F32, tag="pb")
                for j in range(4):
                    nc.tensor.transpose(pb[:, j, :],
                                        pt[0:75, 128 * j:128 * (j + 1)],
                                        ident[0:75, 0:75])

                # output staging with transforms (ob col layout:
                # [aM mu_lo s_lo mu_la s_la rho | aY muY sY | aF muF sF |
                #  aFA muFA sFA])
                ob = obp.tile([128, 4, 75], F32, tag="ob")
                nc.vector.tensor_copy(          # mu_lo, mu_la
                    rap(ob[:, :, 5:6], (75, 4), (10, 2), (1, 5)),
                    rap(pb[:, :, 20:21], (75, 4), (10, 2), (1, 5)))
                nc.scalar.activation(          # s_lo, s_la
                    rap(ob[:, :, 10:11], (75, 4), (10, 2), (1, 5)),
                    rap(pb[:, :, 25:26], (75, 4), (10, 2), (1, 5)), AF.Exp)
                nc.scalar.activation(ob[:, :, 25:30], pb[:, :, 40:45],
                                     AF.Tanh)  # rho
                nc.vector.tensor_copy(          # mu y/f/fa
                    rap(ob[:, :, 35:36], (75, 4), (15, 3), (1, 5)),
                    rap(pb[:, :, 45:46], (75, 4), (10, 3), (1, 5)))
                nc.scalar.activation(          # sig y/f/fa
                    rap(ob[:, :, 40:41], (75, 4), (15, 3), (1, 5)),
                    rap(pb[:, :, 50:51], (75, 4), (10, 3), (1, 5)), AF.Exp)
                e_ = sm.tile([128, 4, 20], F32, tag="e_")
                nc.scalar.activation(e_, pb[:, :, 0:20], AF.Exp)
                ssum = sm.tile([128, 4, 4], F32, tag="ssum")
                nc.vector.tensor_reduce(
                    ssum, e_.rearrange("p t (a b) -> p t a b", a=4), X, OP.add)
                rcp = sm.tile([128, 4, 4], F32, tag="rcp")
                nc.vector.reciprocal(rcp, ssum)
                nc.gpsimd.tensor_tensor(   # alphas merge
                    rap(ob[:, :, 0:1], (75, 4), (1, 5)),
                    e_[:, :, 0:5],
                    rap(rcp[:, :, 0:1], (4, 4), (0, 5)), op=OP.mult)
                nc.gpsimd.tensor_tensor(   # alphas y/f/fa
                    rap(ob[:, :, 30:31], (75, 4), (15, 3), (1, 5)),
                    e_[:, :, 5:20].rearrange("p t (a b) -> p t a b", a=3),
                    rap(rcp[:, :, 1:2], (4, 4), (1, 3), (0, 5)), op=OP.mult)

                # selection: onehot(argmax(logits+gumbel))
                q = sm.tile([128, 4, 20], F32, tag="q")
                nc.vector.tensor_add(q, pb[:, :, 0:20], gmb[:, t, :, :])
                qmax = sm.tile([128, 4, 4], F32, tag="qmax")
                nc.vector.tensor_reduce(
                    qmax, q.rearrange("p t (a b) -> p t a b", a=4), X, OP.max)
                oh = sm.tile([128, 4, 20], F32, tag="oh")
                nc.vector.tensor_tensor(
                    oh.rearrange("p t (a b) -> p t a b", a=4),
                    q.rearrange("p t (a b) -> p t a b", a=4),
                    rap(qmax[:, :, 0:1], (4, 4), (1, 4), (0, 5)),
                    op=OP.is_equal)
                # masked params (sigmas already exp'd in ob)
                # mm: [mu_lo s_lo mu_la s_la | rho] x k (ob natural order)
                mm = sm.tile([128, 4, 25], F32, tag="mm")
                nc.vector.tensor_tensor(
                    mm[:, :, 0:20].rearrange("p t (a b) -> p t a b", a=4),
                    ob[:, :, 5:25].rearrange("p t (a b) -> p t a b", a=4),
                    rap(oh[:, :, 0:1], (20, 4), (0, 4), (1, 5)), op=OP.mult)
                nc.vector.tensor_tensor(mm[:, :, 20:25], ob[:, :, 25:30],
                                        oh[:, :, 0:5], op=OP.mult)
                # mo: [muY sY muF sF muFA sFA] x k
                mo = sm.tile([128, 4, 30], F32, tag="mo")
                nc.vector.tensor_tensor(   # mus
                    rap(mo[:, :, 0:1], (30, 4), (10, 3), (1, 5)),
                    rap(ob[:, :, 35:36], (75, 4), (15, 3), (1, 5)),
                    rap(oh[:, :, 5:6], (20, 4), (5, 3), (1, 5)), op=OP.mult)
                nc.vector.tensor_tensor(   # sigmas
                    rap(mo[:, :, 5:6], (30, 4), (10, 3), (1, 5)),
                    rap(ob[:, :, 40:41], (75, 4), (15, 3), (1, 5)),
                    rap(oh[:, :, 5:6], (20, 4), (5, 3), (1, 5)), op=OP.mult)
                # sel: [mu_lo s_lo mu_la s_la rho muY sY muF sF muFA sFA]
                sel = sm.tile([128, 4, 12], F32, tag="sel")
                nc.vector.tensor_reduce(
                    sel[:, :, 0:4],
                    mm[:, :, 0:20].rearrange("p t (a b) -> p t a b", a=4),
                    X, OP.add)
                nc.vector.tensor_reduce(sel[:, :, 4:5], mm[:, :, 20:25],
                                        X, OP.add)
                nc.vector.tensor_reduce(
                    sel[:, :, 5:11],
                    mo.rearrange("p t (a b) -> p t a b", a=6), X, OP.add)
                # samples -> next cond (layout B)
                r2 = sm.tile([128, 4, 1], F32, tag="r2")
                nc.vector.tensor_mul(r2, sel[:, :, 4:5], sel[:, :, 4:5])
                sq = sm.tile([128, 4, 1], F32, tag="sq")
                nc.vector.tensor_scalar(sq, r2, -0.5, 1.0, OP.mult, OP.add)
                u1 = sm.tile([128, 4, 1], F32, tag="u1")
                nc.vector.tensor_mul(u1, sel[:, :, 4:5], nrb[:, t, :, 0:1])
                u2 = sm.tile([128, 4, 1], F32, tag="u2")
                nc.vector.tensor_mul(u2, sq, nrb[:, t, :, 5:6])
                mix = sm.tile([128, 4, 1], F32, tag="mix")
                nc.vector.tensor_add(mix, u1, u2)
                cb = sm.tile([128, 4, 8], F32, tag="cb")
                nc.vector.memset(cb[:, :, 5:6], 1.0)
                tm = sm.tile([128, 4, 2], F32, tag="tm")
                nc.vector.tensor_mul(tm, rap(sel[:, :, 1:2], (12, 4), (2, 2)),
                                     nrb[:, t, :, 0:2])
                nc.vector.tensor_add(cb[:, :, 0:2],
                                     rap(sel[:, :, 0:1], (12, 4), (2, 2)), tm)
                w_ = sm.tile([128, 4, 1], F32, tag="w_")
                nc.vector.tensor_mul(w_, sel[:, :, 3:4], mix)
                nc.vector.tensor_add(cb[:, :, 1:2], sel[:, :, 2:3], w_)
                to = sm.tile([128, 4, 3], F32, tag="to")
                nc.vector.tensor_mul(to, rap(sel[:, :, 6:7], (12, 4), (2, 3)),
                                     nrb[:, t, :, 2:5])
                nc.vector.tensor_add(cb[:, :, 2:5],
                                     rap(sel[:, :, 5:6], (12, 4), (2, 3)), to)

                if t + 1 < T:
                    cps = psc.tile([128, 512], F32, tag="cps")
                    for j in range(4):
                        nc.tensor.transpose(
                            cps[0:6, 128 * j:128 * (j + 1)],
                            cb[:, j, 0:6], ident)
                    ct = ctp.tile([128, 512], F32R, tag="ct")
                    nc.scalar.activation(ct[0:6, :], cps[0:6, :], AF.Copy)

                oap = d_out[:, :, t, :]
                oap2 = bass.AP(oap.tensor, oap.offset,
                               [oap.ap[1], oap.ap[0], oap.ap[2]])
                nc.sync.dma_start(out=oap2, in_=ob)

    nc.finalize()
    _CACHE["nc"] = nc
    return nc


# ----------------------------------------------------------------- kernel ---
def kernel(**inputs):
    from concourse.bass_utils import run_bass_kernel_spmd

    assert int(inputs["steps_n"]) == T
    conditions = np.asarray(inputs["conditions"], np.float32)
    state_h = np.asarray(inputs["state_h"], np.float32)
    state_c = np.asarray(inputs["state_c"], np.float32)
    Wce, Wcomb, zoff, W_all, b_all = _fold(inputs)
    gum, nrm = _noise()
    nc = _build()

    wcb_h = np.ascontiguousarray(
        Wcomb.reshape(2, 128, 8, 128).transpose(1, 0, 2, 3))  # [128,2,8,128]
    wce_h = np.zeros((6, 8 * T, 128), np.float32)
    for t in range(T):
        for s in range(2):
            for u in range(4):
                sl = _slice_id(s, u)
                blk = 8 * t + 4 * s + u
                wce_h[0:5, blk, :] = Wce[:, 128 * sl:128 * (sl + 1)]
                wce_h[5, blk, :] = zoff[t, 128 * sl:128 * (sl + 1)]
    wal_h = np.ascontiguousarray(
        (0.5 * W_all).astype(np.float16).reshape(2, 128, 75).transpose(
            1, 0, 2))                                      # [128, 2, 75]
    bal_h = np.ascontiguousarray(b_all.reshape(75, 1))

    in_maps = []
    for c in range(NCORES):
        b0 = c * BC
        sht = np.ascontiguousarray(
            state_h[b0:b0 + BC].T.reshape(2, 128, BC).transpose(1, 0, 2))
        c0t = np.ascontiguousarray(
            state_c[b0:b0 + BC].T.reshape(2, 128, BC).transpose(
                1, 0, 2)).astype(np.float16)
        gmb = np.ascontiguousarray(
            gum[:, b0:b0 + BC, :].reshape(T, NB, 128, 20).transpose(
                2, 0, 1, 3))
        nrb = np.ascontiguousarray(
            nrm[:, b0:b0 + BC, :].reshape(T, NB, 128, 6).transpose(
                2, 0, 1, 3))
        ct0 = np.zeros((128, 512), np.float32)
        ct0[0:5, :] = conditions[b0:b0 + BC, 0, :].T
        ct0[5, :] = 1.0
        in_maps.append(dict(
            sht=sht, wcb=wcb_h, c0t=c0t, wce=wce_h,
            wal=wal_h, bal=bal_h,
            gmb=gmb, nrb=nrb, ct0=ct0))

    res = run_bass_kernel_spmd(nc, in_maps, core_ids=list(range(NCORES)))
    _CACHE["last_result"] = res
    full = np.concatenate(
        [np.asarray(res.results[c]["out"]).reshape(BC, T, 75)
         for c in range(NCORES)], 0)
    return (np.ascontiguousarray(full[:, :, 0:30]),
            np.ascontiguousarray(full[:, :, 30:45]),
            np.ascontiguousarray(full[:, :, 45:60]),
            np.ascontiguousarray(full[:, :, 60:75]))


# revision 11
# speedup vs baseline: 1.2843x; 1.2843x over previous
"""Trainium2 Bass kernel for nn_Decoder_9775345565829.

Data-parallel over 8 NeuronCores (B=4096 -> 512/core). Sequential T=20 scan.

Math refactoring (exact algebra):
  z_t = cond_t @ Wce + zbase + zoff[t]     (gate order permuted to [i,f,o,g])
    Wce = We1@We2@Wk[:E]  (rank-5; aug row 5 carries zoff[t])
    zbase = state_h @ (Wk[E:E+H] + Wr)     (step-independent, host-precomputed)
    zoff[t] = cumsum(Wk[E+H:])[t] + bl + (be1@We2+be2)@Wk[:E]
  sigmoid-free gates (single ACT table set "exp_and_others"):
    T* = tanh(z*/2);  2c = (1+Tf)*c0 + (1+Ti)*tanh(zg);  tanh(c) via scale=0.5
    2h = (1+To)*tanh(c);  heads use W_all/2 so p = h @ W_all + b_all
  sampling: host-precomputed threefry gumbel/normal noise; categorical =
  onehot(argmax(logits+gumbel)); sqrt(1-rho^2) by poly (|rho| small).

Device: gates feature-on-partition (8 slices x [128, 512b]); heads/sampling
batch-on-partition via PE transposes.
"""
import os

os.environ.setdefault("JAX_PLATFORMS", "axon,cpu")

import numpy as np

B, T, P, H, E, K, COND = 4096, 20, 20, 256, 128, 5, 5
NCORES, BC, NB = 8, 512, 4
H4 = 4 * H

_CACHE = {}


# ------------------------------------------------------------------ noise ---
def _noise():
    """Replay the reference's jax.random key tree (threefry is platform-
    deterministic). gum [T,B,20] head-major (m,y,f,fa); nrm [T,B,6] cols
    [z1, 0, zy, zf, zfa, z2]."""
    if "noise" in _CACHE:
        return _CACHE["noise"]
    import jax

    try:
        dev = jax.devices("cpu")[0]
    except RuntimeError:
        dev = jax.devices()[0]
    gum = np.zeros((T, B, 20), np.float32)
    nrm = np.zeros((T, B, 6), np.float32)
    with jax.default_device(dev):
        key = jax.random.key(42)
        for t in range(T):
            key, km, ky, kf, kfa = jax.random.split(key, 5)
            k1, k2 = jax.random.split(km)
            gum[t, :, 0:5] = np.asarray(jax.random.gumbel(k1, (B, K)))
            zm = np.asarray(jax.random.normal(k2, (B, 2)))
            nrm[t, :, 0], nrm[t, :, 5] = zm[:, 0], zm[:, 1]
            for j, kh in enumerate((ky, kf, kfa)):
                k1, k2 = jax.random.split(kh)
                gum[t, :, 5 + 5 * j:10 + 5 * j] = np.asarray(
                    jax.random.gumbel(k1, (B, K)))
                nrm[t, :, 2 + j] = np.asarray(
                    jax.random.normal(k2, (B, 1)))[:, 0]
    _CACHE["noise"] = (gum, nrm)
    return gum, nrm


# ---------------------------------------------------------------- weights ---
def _fold(inp):
    f32 = lambda k: np.asarray(inp[k], np.float32)  # noqa: E731
    We1, be1, We2, be2 = f32("We1"), f32("be1"), f32("We2"), f32("be2")
    Wk, Wr, bl = f32("Wk"), f32("Wr"), f32("bl")
    Wm, bm = f32("Wm"), f32("bm")
    Wy, by = f32("Wy"), f32("by")
    Wf, bf = f32("Wf"), f32("bf")
    Wfa, bfa = f32("Wfadj"), f32("bfadj")
    idx = np.arange(H4).reshape(4, H)
    gp = np.concatenate([idx[0], idx[1], idx[3], idx[2]])  # i,f,g,o -> i,f,o,g
    Wk_e, Wk_h, Wk_p = Wk[:E], Wk[E:E + H], Wk[E + H:]
    Wce = (We1 @ We2 @ Wk_e)[:, gp].astype(np.float32)
    Wcomb = (Wk_h + Wr)[:, gp].astype(np.float32)
    embb = be1 @ We2 + be2
    zoff = (np.cumsum(Wk_p, 0)[np.arange(T).clip(0, P - 1)] + bl
            + embb @ Wk_e)[:, gp].astype(np.float32)
    half = np.concatenate([np.full(3 * H, 0.5, np.float32),
                           np.ones(H, np.float32)])
    Wce *= half
    Wcomb *= half
    zoff *= half
    W_all = np.zeros((H, 75), np.float32)
    b_all = np.zeros((75,), np.float32)
    W_all[:, 0:5], b_all[0:5] = Wm[:, 0:5], bm[0:5]
    W_all[:, 5:10], b_all[5:10] = Wy[:, 0:5], by[0:5]
    W_all[:, 10:15], b_all[10:15] = Wf[:, 0:5], bf[0:5]
    W_all[:, 15:20], b_all[15:20] = Wfa[:, 0:5], bfa[0:5]
    W_all[:, 20:45], b_all[20:45] = Wm[:, 5:30], bm[5:30]
    W_all[:, 45:55], b_all[45:55] = Wy[:, 5:15], by[5:15]
    W_all[:, 55:65], b_all[55:65] = Wf[:, 5:15], bf[5:15]
    W_all[:, 65:75], b_all[65:75] = Wfa[:, 5:15], bfa[5:15]
    return Wce, Wcomb, zoff, W_all, b_all


def _slice_id(s, u):  # wave s slot u -> feature slice (order [i,f,o,g])
    return (s, 2 + s, 4 + s, 6 + s)[u]


# ------------------------------------------------------------ bass module ---
def _build():
    if "nc" in _CACHE:
        return _CACHE["nc"]
    import concourse.bass as bass
    import concourse.mybir as mybir
    from concourse import bacc
    from concourse.tile import TileContext
    from concourse.masks import make_identity

    F32, F16, F32R = mybir.dt.float32, mybir.dt.float16, mybir.dt.float32r
    AF, OP = mybir.ActivationFunctionType, mybir.AluOpType
    X = mybir.AxisListType.X

    def rap(src, *dims):
        """new AP reusing src's tensor/partition-dim/offset, with the free
        dims replaced by explicit (step, count) pairs (element units)."""
        return bass.AP(src.tensor, src.offset,
                       [list(src.ap[0])] + [[s_, c_] for (s_, c_) in dims])

    nc = bacc.Bacc(None, target_bir_lowering=False)
    d_sht = nc.dram_tensor("sht", [128, 2, 512], F32R, kind="ExternalInput")
    d_wcb = nc.dram_tensor("wcb", [128, 2, 8, 128], F32R, kind="ExternalInput")
    d_c0t = nc.dram_tensor("c0t", [128, 2, 512], F16, kind="ExternalInput")
    d_wce = nc.dram_tensor("wce", [6, 8 * T, 128], F32R, kind="ExternalInput")
    d_wal = nc.dram_tensor("wal", [128, 2, 75], F16, kind="ExternalInput")
    d_bal = nc.dram_tensor("bal", [75, 1], F32, kind="ExternalInput")
    d_gmb = nc.dram_tensor("gmb", [128, T, 4, 20], F32, kind="ExternalInput")
    d_nrb = nc.dram_tensor("nrb", [128, T, 4, 6], F32, kind="ExternalInput")
    d_ct0 = nc.dram_tensor("ct0", [128, 512], F32R, kind="ExternalInput")
    d_out = nc.dram_tensor("out", [4, 128, T, 75], F32, kind="ExternalOutput")

    with TileContext(nc) as tc:
        with tc.tile_pool(name="const", bufs=1) as cp, \
             tc.tile_pool(name="gw", bufs=2) as gw, \
             tc.tile_pool(name="sm", bufs=3) as sm, \
             tc.tile_pool(name="obp", bufs=3) as obp, \
             tc.tile_pool(name="ctp", bufs=2) as ctp, \
             tc.tile_pool(name="psz", bufs=4, space="PSUM") as psz, \
             tc.tile_pool(name="psp", bufs=1, space="PSUM") as psp, \
             tc.tile_pool(name="psb", bufs=2, space="PSUM") as psb, \
             tc.tile_pool(name="psc", bufs=1, space="PSUM") as psc:

            sht = cp.tile([128, 2, 512], F32R)
            nc.sync.dma_start(out=sht, in_=d_sht[:, :, :])
            wcb = cp.tile([128, 2, 8, 128], F32R)
            nc.sync.dma_start(out=wcb, in_=d_wcb[:, :, :, :])
            c0t = cp.tile([128, 2, 512], F16)
            nc.sync.dma_start(out=c0t, in_=d_c0t[:, :, :])
            wce = cp.tile([128, 8 * T, 128], F32R)
            nc.sync.dma_start(out=wce[0:6, :, :], in_=d_wce[:, :, :])
            wal = cp.tile([128, 2, 75], F16)
            nc.sync.dma_start(out=wal, in_=d_wal[:, :, :])
            bal = cp.tile([128, 1], F32)
            nc.sync.dma_start(out=bal[0:75, :], in_=d_bal[:, :])
            gmb = cp.tile([128, T, 4, 20], F32)
            nc.sync.dma_start(out=gmb, in_=d_gmb[:, :, :, :])
            nrb = cp.tile([128, T, 4, 6], F32)
            nc.sync.dma_start(out=nrb, in_=d_nrb[:, :, :, :])
            ident = cp.tile([128, 128], F32)
            make_identity(nc, ident)

            ct = ctp.tile([128, 512], F32R, tag="ct")
            nc.sync.dma_start(out=ct, in_=d_ct0[:, :])

            for t in range(T):
                h2 = gw.tile([128, 2, 512], F16, tag="h2")

                def flush(pend):
                    # software-pipelined tail of a gate wave: tanh(c) and 2h
                    cs_, thB_, s_, bs_ = pend
                    tcn = gw.tile([128, 256], F16, tag="tcn")
                    nc.scalar.activation(tcn, cs_, AF.Tanh, scale=0.5)
                    nc.vector.scalar_tensor_tensor(
                        h2[:, s_, bs_], thB_[:, 0, :], 1.0, tcn,
                        OP.add, OP.mult)

                pend = None
                for s in range(2):
                    for b in range(2):
                        bs = slice(256 * b, 256 * (b + 1))
                        ztA = psz.tile([128, 2, 256], F32, tag="zt")  # i,f
                        ztB = psz.tile([128, 2, 256], F32, tag="zt")  # o,g
                        for zt_, us in ((ztA, (0, 1)), (ztB, (2, 3))):
                            for u2, u in enumerate(us):
                                sl = _slice_id(s, u)
                                # zbase accumulation first (no cond dep)
                                for k2 in range(2):
                                    nc.tensor.matmul(
                                        zt_[:, u2, :],
                                        lhsT=wcb[:, k2, sl, :],
                                        rhs=sht[:, k2, bs],
                                        start=(k2 == 0), stop=False)
                                nc.tensor.matmul(
                                    zt_[:, u2, :],
                                    lhsT=wce[0:6, 8 * t + 4 * s + u, :],
                                    rhs=ct[0:6, bs],
                                    start=False, stop=True)
                        thA = gw.tile([128, 2, 256], F16, tag="thA")
                        nc.scalar.activation(thA, ztA, AF.Tanh)
                        thB = gw.tile([128, 2, 256], F16, tag="thB")
                        nc.scalar.activation(thB, ztB, AF.Tanh)
                        a1 = gw.tile([128, 256], F16, tag="a1")
                        nc.vector.scalar_tensor_tensor(
                            a1, thA[:, 1, :], 1.0, c0t[:, s, bs],
                            OP.add, OP.mult)
                        a2 = gw.tile([128, 256], F16, tag="a2")
                        nc.vector.scalar_tensor_tensor(
                            a2, thA[:, 0, :], 1.0, thB[:, 1, :],
                            OP.add, OP.mult)
                        if pend is not None:
                            flush(pend)
                        cs = gw.tile([128, 256], F16, tag="cs")
                        nc.gpsimd.tensor_add(cs, a1, a2)  # 2c
                        pend = (cs, thB, s, bs)
                flush(pend)

                # heads
                pp = psp.tile([128, 512], F32, tag="pp")
                for k2 in range(2):
                    nc.tensor.matmul(pp[0:75, :], lhsT=wal[:, k2, :],
                                     rhs=h2[:, k2, :],
                                     start=(k2 == 0), stop=(k2 == 1))
                pt = sm.tile([128, 512], F32, tag="pt")
                nc.scalar.activation(pt[0:75, :], pp[0:75, :], AF.Identity,
                                     bias=bal[0:75, :])
                pb = psb.tile([128, 4, 75], F32, tag="pb")
                for j in range(4):
                    nc.tensor.transpose(pb[:, j, :],
                                        pt[0:75, 128 * j:128 * (j + 1)],
                                        ident[0:75, 0:75])

                # output staging with transforms (ob col layout:
                # [aM mu_lo s_lo mu_la s_la rho | aY muY sY | aF muF sF |
                #  aFA muFA sFA])
                ob = obp.tile([128, 4, 75], F32, tag="ob")
                nc.vector.tensor_copy(          # mu_lo, mu_la
                    rap(ob[:, :, 5:6], (75, 4), (10, 2), (1, 5)),
                    rap(pb[:, :, 20:21], (75, 4), (10, 2), (1, 5)))
                nc.scalar.activation(          # s_lo, s_la
                    rap(ob[:, :, 10:11], (75, 4), (10, 2), (1, 5)),
                    rap(pb[:, :, 25:26], (75, 4), (10, 2), (1, 5)), AF.Exp)
                nc.scalar.activation(ob[:, :, 25:30], pb[:, :, 40:45],
                                     AF.Tanh)  # rho
                nc.vector.tensor_copy(          # mu y/f/fa
                    rap(ob[:, :, 35:36], (75, 4), (15, 3), (1, 5)),
                    rap(pb[:, :, 45:46], (75, 4), (10, 3), (1, 5)))
                nc.scalar.activation(          # sig y/f/fa
                    rap(ob[:, :, 40:41], (75, 4), (15, 3), (1, 5)),
                    rap(pb[:, :, 50:51], (75, 4), (10, 3), (1, 5)), AF.Exp)
                e_ = sm.tile([128, 4, 20], F32, tag="e_")
                nc.scalar.activation(e_, pb[:, :, 0:20], AF.Exp)
                ssum = sm.tile([128, 4, 4], F32, tag="ssum")
                nc.vector.tensor_reduce(
                    ssum, e_.rearrange("p t (a b) -> p t a b", a=4), X, OP.add)
                rcp = sm.tile([128, 4, 4], F32, tag="rcp")
                nc.vector.reciprocal(rcp, ssum)
                nc.gpsimd.tensor_tensor(   # alphas merge
                    rap(ob[:, :, 0:1], (75, 4), (1, 5)),
                    e_[:, :, 0:5],
                    rap(rcp[:, :, 0:1], (4, 4), (0, 5)), op=OP.mult)
                nc.gpsimd.tensor_tensor(   # alphas y/f/fa
                    rap(ob[:, :, 30:31], (75, 4), (15, 3), (1, 5)),
                    e_[:, :, 5:20].rearrange("p t (a b) -> p t a b", a=3),
                    rap(rcp[:, :, 1:2], (4, 4), (1, 3), (0, 5)), op=OP.mult)

                # selection: onehot(argmax(logits+gumbel))
                q = sm.tile([128, 4, 20], F32, tag="q")
                nc.vector.tensor_add(q, pb[:, :, 0:20], gmb[:, t, :, :])
                qmax = sm.tile([128, 4, 4], F32, tag="qmax")
                nc.vector.tensor_reduce(
                    qmax, q.rearrange("p t (a b) -> p t a b", a=4), X, OP.max)
                oh = sm.tile([128, 4, 20], F32, tag="oh")
                nc.vector.tensor_tensor(
                    oh.rearrange("p t (a b) -> p t a b", a=4),
                    q.rearrange("p t (a b) -> p t a b", a=4),
                    rap(qmax[:, :, 0:1], (4, 4), (1, 4), (0, 5)),
                    op=OP.is_equal)
                # masked params (sigmas already exp'd in ob)
                # mm: [mu_lo s_lo mu_la s_la | rho] x k (ob natural order)
                mm = sm.tile([128, 4, 25], F32, tag="mm")
                nc.vector.tensor_tensor(
                    mm[:, :, 0:20].rearrange("p t (a b) -> p t a b", a=4),
                    ob[:, :, 5:25].rearrange("p t (a b) -> p t a b", a=4),
                    rap(oh[:, :, 0:1], (20, 4), (0, 4), (1, 5)), op=OP.mult)
                nc.vector.tensor_tensor(mm[:, :, 20:25], ob[:, :, 25:30],
                                        oh[:, :, 0:5], op=OP.mult)
                # mo: [muY sY muF sF muFA sFA] x k
                mo = sm.tile([128, 4, 30], F32, tag="mo")
                nc.vector.tensor_tensor(   # mus
                    rap(mo[:, :, 0:1], (30, 4), (10, 3), (1, 5)),
                    rap(ob[:, :, 35:36], (75, 4), (15, 3), (1, 5)),
                    rap(oh[:, :, 5:6], (20, 4), (5, 3), (1, 5)), op=OP.mult)
                nc.vector.tensor_tensor(   # sigmas
                    rap(mo[:, :, 5:6], (30, 4), (10, 3), (1, 5)),
                    rap(ob[:, :, 40:41], (75, 4), (15, 3), (1, 5)),
                    rap(oh[:, :, 5:6], (20, 4), (5, 3), (1, 5)), op=OP.mult)
                # sel: [mu_lo s_lo mu_la s_la rho muY sY muF sF muFA sFA]
                sel = sm.tile([128, 4, 12], F32, tag="sel")
                nc.vector.tensor_reduce(
                    sel[:, :, 0:4],
                    mm[:, :, 0:20].rearrange("p t (a b) -> p t a b", a=4),
                    X, OP.add)
                nc.vector.tensor_reduce(sel[:, :, 4:5], mm[:, :, 20:25],
                                        X, OP.add)
                nc.vector.tensor_reduce(
                    sel[:, :, 5:11],
                    mo.rearrange("p t (a b) -> p t a b", a=6), X, OP.add)
                # samples -> next cond (layout B)
                r2 = sm.tile([128, 4, 1], F32, tag="r2")
                nc.vector.tensor_mul(r2, sel[:, :, 4:5], sel[:, :, 4:5])
                sq = sm.tile([128, 4, 1], F32, tag="sq")
                nc.vector.tensor_scalar(sq, r2, -0.5, 1.0, OP.mult, OP.add)
                u1 = sm.tile([128, 4, 1], F32, tag="u1")
                nc.vector.tensor_mul(u1, sel[:, :, 4:5], nrb[:, t, :, 0:1])
                u2 = sm.tile([128, 4, 1], F32, tag="u2")
                nc.vector.tensor_mul(u2, sq, nrb[:, t, :, 5:6])
                mix = sm.tile([128, 4, 1], F32, tag="mix")
                nc.vector.tensor_add(mix, u1, u2)
                cb = sm.tile([128, 4, 8], F32, tag="cb")
                nc.vector.memset(cb[:, :, 5:6], 1.0)
                tm = sm.tile([128, 4, 2], F32, tag="tm")
                nc.vector.tensor_mul(tm, rap(sel[:, :, 1:2], (12, 4), (2, 2)),
                                     nrb[:, t, :, 0:2])
                nc.vector.tensor_add(cb[:, :, 0:2],
                                     rap(sel[:, :, 0:1], (12, 4), (2, 2)), tm)
                w_ = sm.tile([128, 4, 1], F32, tag="w_")
                nc.vector.tensor_mul(w_, sel[:, :, 3:4], mix)
                nc.vector.tensor_add(cb[:, :, 1:2], sel[:, :, 2:3], w_)
                to = sm.tile([128, 4, 3], F32, tag="to")
                nc.vector.tensor_mul(to, rap(sel[:, :, 6:7], (12, 4), (2, 3)),
                                     nrb[:, t, :, 2:5])
                nc.vector.tensor_add(cb[:, :, 2:5],
                                     rap(sel[:, :, 5:6], (12, 4), (2, 3)), to)

                if t + 1 < T:
                    cps = psc.tile([128, 512], F32, tag="cps")
                    for j in range(4):
                        nc.tensor.transpose(
                            cps[0:6, 128 * j:128 * (j + 1)],
                            cb[:, j, 0:6], ident)
                    ct = ctp.tile([128, 512], F32R, tag="ct")
                    nc.scalar.activation(ct[0:6, :], cps[0:6, :], AF.Copy)

                oap = d_out[:, :, t, :]
                oap2 = bass.AP(oap.tensor, oap.offset,
                               [oap.ap[1], oap.ap[0], oap.ap[2]])
                nc.sync.dma_start(out=oap2, in_=ob)

    nc.finalize()
    _CACHE["nc"] = nc
    return nc


# ----------------------------------------------------------------- kernel ---
def kernel(**inputs):
    from concourse.bass_utils import run_bass_kernel_spmd

    assert int(inputs["steps_n"]) == T
    conditions = np.asarray(inputs["conditions"], np.float32)
    state_h = np.asarray(inputs["state_h"], np.float32)
    state_c = np.asarray(inputs["state_c"], np.float32)
    Wce, Wcomb, zoff, W_all, b_all = _fold(inputs)
    gum, nrm = _noise()
    nc = _build()

    wcb_h = np.ascontiguousarray(
        Wcomb.reshape(2, 128, 8, 128).transpose(1, 0, 2, 3))  # [128,2,8,128]
    wce_h = np.zeros((6, 8 * T, 128), np.float32)
    for t in range(T):
        for s in range(2):
            for u in range(4):
                sl = _slice_id(s, u)
                blk = 8 * t + 4 * s + u
                wce_h[0:5, blk, :] = Wce[:, 128 * sl:128 * (sl + 1)]
                wce_h[5, blk, :] = zoff[t, 128 * sl:128 * (sl + 1)]
    wal_h = np.ascontiguousarray(
        (0.5 * W_all).astype(np.float16).reshape(2, 128, 75).transpose(
            1, 0, 2))                                      # [128, 2, 75]
    bal_h = np.ascontiguousarray(b_all.reshape(75, 1))

    in_maps = []
    for c in range(NCORES):
        b0 = c * BC
        sht = np.ascontiguousarray(
            state_h[b0:b0 + BC].T.reshape(2, 128, BC).transpose(1, 0, 2))
        c0t = np.ascontiguousarray(
            state_c[b0:b0 + BC].T.reshape(2, 128, BC).transpose(
                1, 0, 2)).astype(np.float16)
        gmb = np.ascontiguousarray(
            gum[:, b0:b0 + BC, :].reshape(T, NB, 128, 20).transpose(
                2, 0, 1, 3))
        nrb = np.ascontiguousarray(
            nrm[:, b0:b0 + BC, :].reshape(T, NB, 128, 6).transpose(
                2, 0, 1, 3))
        ct0 = np.zeros((128, 512), np.float32)
        ct0[0:5, :] = conditions[b0:b0 + BC, 0, :].T
        ct0[5, :] = 1.0
        in_maps.append(dict(
            sht=sht, wcb=wcb_h, c0t=c0t, wce=wce_h,
            wal=wal_h, bal=bal_h,
            gmb=gmb, nrb=nrb, ct0=ct0))

    res = run_bass_kernel_spmd(nc, in_maps, core_ids=list(range(NCORES)))
    _CACHE["last_result"] = res
    full = np.concatenate(
        [np.asarray(res.results[c]["out"]).reshape(BC, T, 75)
         for c in range(NCORES)], 0)
    return (np.ascontiguousarray(full[:, :, 0:30]),
            np.ascontiguousarray(full[:, :, 30:45]),
            np.ascontiguousarray(full[:, :, 45:60]),
            np.ascontiguousarray(full[:, :, 60:75]))


# revision 13
# speedup vs baseline: 1.3114x; 1.0211x over previous
"""Trainium2 Bass kernel for nn_Decoder_9775345565829.

Data-parallel over 8 NeuronCores (B=4096 -> 512/core). Sequential T=20 scan.

Math refactoring (exact algebra):
  z_t = cond_t @ Wce + zbase + zoff[t]     (gate order permuted to [i,f,o,g])
    Wce = We1@We2@Wk[:E]  (rank-5; aug row 5 carries zoff[t])
    zbase = state_h @ (Wk[E:E+H] + Wr)     (step-independent, host-precomputed)
    zoff[t] = cumsum(Wk[E+H:])[t] + bl + (be1@We2+be2)@Wk[:E]
  sigmoid-free gates (single ACT table set "exp_and_others"):
    T* = tanh(z*/2);  2c = (1+Tf)*c0 + (1+Ti)*tanh(zg);  tanh(c) via scale=0.5
    2h = (1+To)*tanh(c);  heads use W_all/2 so p = h @ W_all + b_all
  sampling: host-precomputed threefry gumbel/normal noise; categorical =
  onehot(argmax(logits+gumbel)); sqrt(1-rho^2) by poly (|rho| small).

Device: gates feature-on-partition (8 slices x [128, 512b]); heads/sampling
batch-on-partition via PE transposes.
"""
import os

os.environ.setdefault("JAX_PLATFORMS", "axon,cpu")

import numpy as np

B, T, P, H, E, K, COND = 4096, 20, 20, 256, 128, 5, 5
NCORES, BC, NB = 8, 512, 4
H4 = 4 * H

_CACHE = {}


# ------------------------------------------------------------------ noise ---
def _noise():
    """Replay the reference's jax.random key tree (threefry is platform-
    deterministic). gum [T,B,20] head-major (m,y,f,fa); nrm [T,B,6] cols
    [z1, 0, zy, zf, zfa, z2]."""
    if "noise" in _CACHE:
        return _CACHE["noise"]
    import jax

    try:
        dev = jax.devices("cpu")[0]
    except RuntimeError:
        dev = jax.devices()[0]
    gum = np.zeros((T, B, 20), np.float32)
    nrm = np.zeros((T, B, 6), np.float32)
    with jax.default_device(dev):
        key = jax.random.key(42)
        for t in range(T):
            key, km, ky, kf, kfa = jax.random.split(key, 5)
            k1, k2 = jax.random.split(km)
            gum[t, :, 0:5] = np.asarray(jax.random.gumbel(k1, (B, K)))
            zm = np.asarray(jax.random.normal(k2, (B, 2)))
            nrm[t, :, 0], nrm[t, :, 5] = zm[:, 0], zm[:, 1]
            for j, kh in enumerate((ky, kf, kfa)):
                k1, k2 = jax.random.split(kh)
                gum[t, :, 5 + 5 * j:10 + 5 * j] = np.asarray(
                    jax.random.gumbel(k1, (B, K)))
                nrm[t, :, 2 + j] = np.asarray(
                    jax.random.normal(k2, (B, 1)))[:, 0]
    _CACHE["noise"] = (gum, nrm)
    return gum, nrm


# ---------------------------------------------------------------- weights ---
def _fold(inp):
    f32 = lambda k: np.asarray(inp[k], np.float32)  # noqa: E731
    We1, be1, We2, be2 = f32("We1"), f32("be1"), f32("We2"), f32("be2")
    Wk, Wr, bl = f32("Wk"), f32("Wr"), f32("bl")
    Wm, bm = f32("Wm"), f32("bm")
    Wy, by = f32("Wy"), f32("by")
    Wf, bf = f32("Wf"), f32("bf")
    Wfa, bfa = f32("Wfadj"), f32("bfadj")
    idx = np.arange(H4).reshape(4, H)
    gp = np.concatenate([idx[0], idx[1], idx[3], idx[2]])  # i,f,g,o -> i,f,o,g
    Wk_e, Wk_h, Wk_p = Wk[:E], Wk[E:E + H], Wk[E + H:]
    Wce = (We1 @ We2 @ Wk_e)[:, gp].astype(np.float32)
    Wcomb = (Wk_h + Wr)[:, gp].astype(np.float32)
    embb = be1 @ We2 + be2
    zoff = (np.cumsum(Wk_p, 0)[np.arange(T).clip(0, P - 1)] + bl
            + embb @ Wk_e)[:, gp].astype(np.float32)
    half = np.concatenate([np.full(3 * H, 0.5, np.float32),
                           np.ones(H, np.float32)])
    Wce *= half
    Wcomb *= half
    zoff *= half
    W_all = np.zeros((H, 75), np.float32)
    b_all = np.zeros((75,), np.float32)
    W_all[:, 0:5], b_all[0:5] = Wm[:, 0:5], bm[0:5]
    W_all[:, 5:10], b_all[5:10] = Wy[:, 0:5], by[0:5]
    W_all[:, 10:15], b_all[10:15] = Wf[:, 0:5], bf[0:5]
    W_all[:, 15:20], b_all[15:20] = Wfa[:, 0:5], bfa[0:5]
    W_all[:, 20:45], b_all[20:45] = Wm[:, 5:30], bm[5:30]
    W_all[:, 45:55], b_all[45:55] = Wy[:, 5:15], by[5:15]
    W_all[:, 55:65], b_all[55:65] = Wf[:, 5:15], bf[5:15]
    W_all[:, 65:75], b_all[65:75] = Wfa[:, 5:15], bfa[5:15]
    return Wce, Wcomb, zoff, W_all, b_all


def _slice_id(s, u):  # wave s slot u -> feature slice (order [i,f,o,g])
    return (s, 2 + s, 4 + s, 6 + s)[u]


# ------------------------------------------------------------ bass module ---
def _build():
    if "nc" in _CACHE:
        return _CACHE["nc"]
    import concourse.bass as bass
    import concourse.mybir as mybir
    from concourse import bacc
    from concourse.tile import TileContext
    from concourse.masks import make_identity

    F32, F16, F32R = mybir.dt.float32, mybir.dt.float16, mybir.dt.float32r
    AF, OP = mybir.ActivationFunctionType, mybir.AluOpType
    X = mybir.AxisListType.X

    def rap(src, *dims):
        """new AP reusing src's tensor/partition-dim/offset, with the free
        dims replaced by explicit (step, count) pairs (element units)."""
        return bass.AP(src.tensor, src.offset,
                       [list(src.ap[0])] + [[s_, c_] for (s_, c_) in dims])

    nc = bacc.Bacc(None, target_bir_lowering=False)
    d_sht = nc.dram_tensor("sht", [128, 2, 512], F32R, kind="ExternalInput")
    d_wcb = nc.dram_tensor("wcb", [128, 2, 8, 128], F32R, kind="ExternalInput")
    d_c0t = nc.dram_tensor("c0t", [128, 2, 512], F16, kind="ExternalInput")
    d_wce = nc.dram_tensor("wce", [6, 8 * T, 128], F32R, kind="ExternalInput")
    d_wal = nc.dram_tensor("wal", [128, 2, 75], F16, kind="ExternalInput")
    d_bal = nc.dram_tensor("bal", [75, 1], F32, kind="ExternalInput")
    d_gmb = nc.dram_tensor("gmb", [128, T, 4, 20], F32, kind="ExternalInput")
    d_nrb = nc.dram_tensor("nrb", [128, T, 4, 6], F32, kind="ExternalInput")
    d_ct0 = nc.dram_tensor("ct0", [128, 512], F32R, kind="ExternalInput")
    d_out = nc.dram_tensor("out", [4, 128, T, 75], F32, kind="ExternalOutput")

    with TileContext(nc) as tc:
        with tc.tile_pool(name="const", bufs=1) as cp, \
             tc.tile_pool(name="gw", bufs=2) as gw, \
             tc.tile_pool(name="sm", bufs=3) as sm, \
             tc.tile_pool(name="obp", bufs=3) as obp, \
             tc.tile_pool(name="ctp", bufs=2) as ctp, \
             tc.tile_pool(name="psz", bufs=4, space="PSUM") as psz, \
             tc.tile_pool(name="psp", bufs=1, space="PSUM") as psp, \
             tc.tile_pool(name="psb", bufs=2, space="PSUM") as psb, \
             tc.tile_pool(name="psc", bufs=1, space="PSUM") as psc:

            sht = cp.tile([128, 2, 512], F32R)
            nc.sync.dma_start(out=sht, in_=d_sht[:, :, :])
            wcb = cp.tile([128, 2, 8, 128], F32R)
            nc.sync.dma_start(out=wcb, in_=d_wcb[:, :, :, :])
            c0t = cp.tile([128, 2, 512], F16)
            nc.sync.dma_start(out=c0t, in_=d_c0t[:, :, :])
            wce = cp.tile([128, 8 * T, 128], F32R)
            nc.sync.dma_start(out=wce[0:6, :, :], in_=d_wce[:, :, :])
            wal = cp.tile([128, 2, 75], F16)
            nc.sync.dma_start(out=wal, in_=d_wal[:, :, :])
            bal = cp.tile([128, 1], F32)
            nc.sync.dma_start(out=bal[0:75, :], in_=d_bal[:, :])
            gmb = cp.tile([128, T, 4, 20], F32)
            nc.sync.dma_start(out=gmb, in_=d_gmb[:, :, :, :])
            nrb = cp.tile([128, T, 4, 6], F32)
            nc.sync.dma_start(out=nrb, in_=d_nrb[:, :, :, :])
            ident = cp.tile([128, 128], F32)
            make_identity(nc, ident)

            ct = ctp.tile([128, 512], F32R, tag="ct")
            nc.sync.dma_start(out=ct, in_=d_ct0[:, :])

            def preload():
                """zbase-accumulation matmuls for the next step: no cond
                dependency, emitted before the current step's sampling so the
                PE fills the sampling-tail gap."""
                lst = []
                for s in range(2):
                    for b in range(2):
                        bs = slice(256 * b, 256 * (b + 1))
                        ztA = psz.tile([128, 2, 256], F32, tag="zt")  # i,f
                        ztB = psz.tile([128, 2, 256], F32, tag="zt")  # o,g
                        for zt_, us in ((ztA, (0, 1)), (ztB, (2, 3))):
                            for u2, u in enumerate(us):
                                sl = _slice_id(s, u)
                                for k2 in range(2):
                                    nc.tensor.matmul(
                                        zt_[:, u2, :],
                                        lhsT=wcb[:, k2, sl, :],
                                        rhs=sht[:, k2, bs],
                                        start=(k2 == 0), stop=False,
                                        skip_group_check=True)
                        lst.append((ztA, ztB, s, b, bs))
                return lst

            zts = preload()
            for t in range(T):
                h2 = gw.tile([128, 2, 512], F16, tag="h2")

                def flush(pend):
                    # software-pipelined tail of a gate wave: tanh(c) and 2h
                    cs_, thB_, s_, bs_ = pend
                    tcn = gw.tile([128, 256], F16, tag="tcn")
                    nc.scalar.activation(tcn, cs_, AF.Tanh, scale=0.5)
                    nc.vector.scalar_tensor_tensor(
                        h2[:, s_, bs_], thB_[:, 0, :], 1.0, tcn,
                        OP.add, OP.mult)

                pend = None
                for ztA, ztB, s, b, bs in zts:
                    for zt_, us in ((ztA, (0, 1)), (ztB, (2, 3))):
                        for u2, u in enumerate(us):
                            nc.tensor.matmul(
                                zt_[:, u2, :],
                                lhsT=wce[0:6, 8 * t + 4 * s + u, :],
                                rhs=ct[0:6, bs],
                                start=False, stop=True,
                                skip_group_check=True)
                if True:
                    for ztA, ztB, s, b, bs in zts:
                        thA = gw.tile([128, 2, 256], F16, tag="thA")
                        nc.scalar.activation(thA, ztA, AF.Tanh)
                        thB = gw.tile([128, 2, 256], F16, tag="thB")
                        nc.scalar.activation(thB, ztB, AF.Tanh)
                        a1 = gw.tile([128, 256], F16, tag="a1")
                        nc.vector.scalar_tensor_tensor(
                            a1, thA[:, 1, :], 1.0, c0t[:, s, bs],
                            OP.add, OP.mult)
                        a2 = gw.tile([128, 256], F16, tag="a2")
                        nc.vector.scalar_tensor_tensor(
                            a2, thA[:, 0, :], 1.0, thB[:, 1, :],
                            OP.add, OP.mult)
                        if pend is not None:
                            flush(pend)
                        cs = gw.tile([128, 256], F16, tag="cs")
                        nc.gpsimd.tensor_add(cs, a1, a2)  # 2c
                        pend = (cs, thB, s, bs)
                flush(pend)

                # heads
                pp = psp.tile([128, 512], F32, tag="pp")
                for k2 in range(2):
                    nc.tensor.matmul(pp[0:75, :], lhsT=wal[:, k2, :],
                                     rhs=h2[:, k2, :],
                                     start=(k2 == 0), stop=(k2 == 1))
                pt = sm.tile([128, 512], F32, tag="pt")
                nc.scalar.activation(pt[0:75, :], pp[0:75, :], AF.Identity,
                                     bias=bal[0:75, :])
                pb = psb.tile([128, 4, 75], F32, tag="pb")
                for j in range(4):
                    nc.tensor.transpose(pb[:, j, :],
                                        pt[0:75, 128 * j:128 * (j + 1)],
                                        ident[0:75, 0:75])

                if t + 1 < T:
                    zts = preload()

                # output staging with transforms (ob col layout:
                # [aM mu_lo s_lo mu_la s_la rho | aY muY sY | aF muF sF |
                #  aFA muFA sFA])
                ob = obp.tile([128, 4, 75], F32, tag="ob")
                nc.vector.tensor_copy(          # mu_lo, mu_la
                    rap(ob[:, :, 5:6], (75, 4), (10, 2), (1, 5)),
                    rap(pb[:, :, 20:21], (75, 4), (10, 2), (1, 5)))
                nc.scalar.activation(          # s_lo, s_la
                    rap(ob[:, :, 10:11], (75, 4), (10, 2), (1, 5)),
                    rap(pb[:, :, 25:26], (75, 4), (10, 2), (1, 5)), AF.Exp)
                nc.scalar.activation(ob[:, :, 25:30], pb[:, :, 40:45],
                                     AF.Tanh)  # rho
                nc.vector.tensor_copy(          # mu y/f/fa
                    rap(ob[:, :, 35:36], (75, 4), (15, 3), (1, 5)),
                    rap(pb[:, :, 45:46], (75, 4), (10, 3), (1, 5)))
                nc.scalar.activation(          # sig y/f/fa
                    rap(ob[:, :, 40:41], (75, 4), (15, 3), (1, 5)),
                    rap(pb[:, :, 50:51], (75, 4), (10, 3), (1, 5)), AF.Exp)
                e_ = sm.tile([128, 4, 20], F32, tag="e_")
                nc.scalar.activation(e_, pb[:, :, 0:20], AF.Exp)
                ssum = sm.tile([128, 4, 4], F32, tag="ssum")
                nc.vector.tensor_reduce(
                    ssum, e_.rearrange("p t (a b) -> p t a b", a=4), X, OP.add)
                rcp = sm.tile([128, 4, 4], F32, tag="rcp")
                nc.vector.reciprocal(rcp, ssum)
                nc.gpsimd.tensor_tensor(   # alphas merge
                    rap(ob[:, :, 0:1], (75, 4), (1, 5)),
                    e_[:, :, 0:5],
                    rap(rcp[:, :, 0:1], (4, 4), (0, 5)), op=OP.mult)
                nc.gpsimd.tensor_tensor(   # alphas y/f/fa
                    rap(ob[:, :, 30:31], (75, 4), (15, 3), (1, 5)),
                    e_[:, :, 5:20].rearrange("p t (a b) -> p t a b", a=3),
                    rap(rcp[:, :, 1:2], (4, 4), (1, 3), (0, 5)), op=OP.mult)

                # selection: onehot(argmax(logits+gumbel))
                q = sm.tile([128, 4, 20], F32, tag="q")
                nc.vector.tensor_add(q, pb[:, :, 0:20], gmb[:, t, :, :])
                qmax = sm.tile([128, 4, 4], F32, tag="qmax")
                nc.vector.tensor_reduce(
                    qmax, q.rearrange("p t (a b) -> p t a b", a=4), X, OP.max)
                oh = sm.tile([128, 4, 20], F32, tag="oh")
                nc.vector.tensor_tensor(
                    oh.rearrange("p t (a b) -> p t a b", a=4),
                    q.rearrange("p t (a b) -> p t a b", a=4),
                    rap(qmax[:, :, 0:1], (4, 4), (1, 4), (0, 5)),
                    op=OP.is_equal)
                # masked params (sigmas already exp'd in ob)
                # mm: [mu_lo s_lo mu_la s_la | rho] x k (ob natural order)
                mm = sm.tile([128, 4, 25], F32, tag="mm")
                nc.vector.tensor_tensor(
                    mm[:, :, 0:20].rearrange("p t (a b) -> p t a b", a=4),
                    ob[:, :, 5:25].rearrange("p t (a b) -> p t a b", a=4),
                    rap(oh[:, :, 0:1], (20, 4), (0, 4), (1, 5)), op=OP.mult)
                nc.vector.tensor_tensor(mm[:, :, 20:25], ob[:, :, 25:30],
                                        oh[:, :, 0:5], op=OP.mult)
                # mo: [muY sY muF sF muFA sFA] x k
                mo = sm.tile([128, 4, 30], F32, tag="mo")
                nc.vector.tensor_tensor(   # mus
                    rap(mo[:, :, 0:1], (30, 4), (10, 3), (1, 5)),
                    rap(ob[:, :, 35:36], (75, 4), (15, 3), (1, 5)),
                    rap(oh[:, :, 5:6], (20, 4), (5, 3), (1, 5)), op=OP.mult)
                nc.vector.tensor_tensor(   # sigmas
                    rap(mo[:, :, 5:6], (30, 4), (10, 3), (1, 5)),
                    rap(ob[:, :, 40:41], (75, 4), (15, 3), (1, 5)),
                    rap(oh[:, :, 5:6], (20, 4), (5, 3), (1, 5)), op=OP.mult)
                # sel: [mu_lo s_lo mu_la s_la rho muY sY muF sF muFA sFA]
                sel = sm.tile([128, 4, 12], F32, tag="sel")
                nc.vector.tensor_reduce(
                    sel[:, :, 0:4],
                    mm[:, :, 0:20].rearrange("p t (a b) -> p t a b", a=4),
                    X, OP.add)
                nc.vector.tensor_reduce(sel[:, :, 4:5], mm[:, :, 20:25],
                                        X, OP.add)
                nc.vector.tensor_reduce(
                    sel[:, :, 5:11],
                    mo.rearrange("p t (a b) -> p t a b", a=6), X, OP.add)
                # samples -> next cond (layout B)
                r2 = sm.tile([128, 4, 1], F32, tag="r2")
                nc.vector.tensor_mul(r2, sel[:, :, 4:5], sel[:, :, 4:5])
                sq = sm.tile([128, 4, 1], F32, tag="sq")
                nc.vector.tensor_scalar(sq, r2, -0.5, 1.0, OP.mult, OP.add)
                u1 = sm.tile([128, 4, 1], F32, tag="u1")
                nc.vector.tensor_mul(u1, sel[:, :, 4:5], nrb[:, t, :, 0:1])
                u2 = sm.tile([128, 4, 1], F32, tag="u2")
                nc.vector.tensor_mul(u2, sq, nrb[:, t, :, 5:6])
                mix = sm.tile([128, 4, 1], F32, tag="mix")
                nc.vector.tensor_add(mix, u1, u2)
                cb = sm.tile([128, 4, 8], F32, tag="cb")
                nc.vector.memset(cb[:, :, 5:6], 1.0)
                tm = sm.tile([128, 4, 2], F32, tag="tm")
                nc.vector.tensor_mul(tm, rap(sel[:, :, 1:2], (12, 4), (2, 2)),
                                     nrb[:, t, :, 0:2])
                nc.vector.tensor_add(cb[:, :, 0:2],
                                     rap(sel[:, :, 0:1], (12, 4), (2, 2)), tm)
                w_ = sm.tile([128, 4, 1], F32, tag="w_")
                nc.vector.tensor_mul(w_, sel[:, :, 3:4], mix)
                nc.vector.tensor_add(cb[:, :, 1:2], sel[:, :, 2:3], w_)
                to = sm.tile([128, 4, 3], F32, tag="to")
                nc.vector.tensor_mul(to, rap(sel[:, :, 6:7], (12, 4), (2, 3)),
                                     nrb[:, t, :, 2:5])
                nc.vector.tensor_add(cb[:, :, 2:5],
                                     rap(sel[:, :, 5:6], (12, 4), (2, 3)), to)

                if t + 1 < T:
                    cps = psc.tile([128, 512], F32, tag="cps")
                    for j in range(4):
                        nc.tensor.transpose(
                            cps[0:6, 128 * j:128 * (j + 1)],
                            cb[:, j, 0:6], ident)
                    ct = ctp.tile([128, 512], F32R, tag="ct")
                    nc.scalar.activation(ct[0:6, :], cps[0:6, :], AF.Copy)

                oap = d_out[:, :, t, :]
                oap2 = bass.AP(oap.tensor, oap.offset,
                               [oap.ap[1], oap.ap[0], oap.ap[2]])
                nc.sync.dma_start(out=oap2, in_=ob)

    nc.finalize()
    _CACHE["nc"] = nc
    return nc


# ----------------------------------------------------------------- kernel ---
def kernel(**inputs):
    from concourse.bass_utils import run_bass_kernel_spmd

    assert int(inputs["steps_n"]) == T
    conditions = np.asarray(inputs["conditions"], np.float32)
    state_h = np.asarray(inputs["state_h"], np.float32)
    state_c = np.asarray(inputs["state_c"], np.float32)
    Wce, Wcomb, zoff, W_all, b_all = _fold(inputs)
    gum, nrm = _noise()
    nc = _build()

    wcb_h = np.ascontiguousarray(
        Wcomb.reshape(2, 128, 8, 128).transpose(1, 0, 2, 3))  # [128,2,8,128]
    wce_h = np.zeros((6, 8 * T, 128), np.float32)
    for t in range(T):
        for s in range(2):
            for u in range(4):
                sl = _slice_id(s, u)
                blk = 8 * t + 4 * s + u
                wce_h[0:5, blk, :] = Wce[:, 128 * sl:128 * (sl + 1)]
                wce_h[5, blk, :] = zoff[t, 128 * sl:128 * (sl + 1)]
    wal_h = np.ascontiguousarray(
        (0.5 * W_all).astype(np.float16).reshape(2, 128, 75).transpose(
            1, 0, 2))                                      # [128, 2, 75]
    bal_h = np.ascontiguousarray(b_all.reshape(75, 1))

    in_maps = []
    for c in range(NCORES):
        b0 = c * BC
        sht = np.ascontiguousarray(
            state_h[b0:b0 + BC].T.reshape(2, 128, BC).transpose(1, 0, 2))
        c0t = np.ascontiguousarray(
            state_c[b0:b0 + BC].T.reshape(2, 128, BC).transpose(
                1, 0, 2)).astype(np.float16)
        gmb = np.ascontiguousarray(
            gum[:, b0:b0 + BC, :].reshape(T, NB, 128, 20).transpose(
                2, 0, 1, 3))
        nrb = np.ascontiguousarray(
            nrm[:, b0:b0 + BC, :].reshape(T, NB, 128, 6).transpose(
                2, 0, 1, 3))
        ct0 = np.zeros((128, 512), np.float32)
        ct0[0:5, :] = conditions[b0:b0 + BC, 0, :].T
        ct0[5, :] = 1.0
        in_maps.append(dict(
            sht=sht, wcb=wcb_h, c0t=c0t, wce=wce_h,
            wal=wal_h, bal=bal_h,
            gmb=gmb, nrb=nrb, ct0=ct0))

    res = run_bass_kernel_spmd(nc, in_maps, core_ids=list(range(NCORES)))
    _CACHE["last_result"] = res
    full = np.concatenate(
        [np.asarray(res.results[c]["out"]).reshape(BC, T, 75)
         for c in range(NCORES)], 0)
    return (np.ascontiguousarray(full[:, :, 0:30]),
            np.ascontiguousarray(full[:, :, 30:45]),
            np.ascontiguousarray(full[:, :, 45:60]),
            np.ascontiguousarray(full[:, :, 60:75]))


# revision 14
# speedup vs baseline: 1.3540x; 1.0325x over previous
"""Trainium2 Bass kernel for nn_Decoder_9775345565829.

Data-parallel over 8 NeuronCores (B=4096 -> 512/core). Sequential T=20 scan.

Math refactoring (exact algebra):
  z_t = cond_t @ Wce + zbase + zoff[t]     (gate order permuted to [i,f,o,g])
    Wce = We1@We2@Wk[:E]  (rank-5; aug row 5 carries zoff[t])
    zbase = state_h @ (Wk[E:E+H] + Wr)     (step-independent, host-precomputed)
    zoff[t] = cumsum(Wk[E+H:])[t] + bl + (be1@We2+be2)@Wk[:E]
  sigmoid-free gates (single ACT table set "exp_and_others"):
    T* = tanh(z*/2);  2c = (1+Tf)*c0 + (1+Ti)*tanh(zg);  tanh(c) via scale=0.5
    2h = (1+To)*tanh(c);  heads use W_all/2 so p = h @ W_all + b_all
  sampling: host-precomputed threefry gumbel/normal noise; categorical =
  onehot(argmax(logits+gumbel)); sqrt(1-rho^2) by poly (|rho| small).

Device: gates feature-on-partition (8 slices x [128, 512b]); heads/sampling
batch-on-partition via PE transposes.
"""
import os

os.environ.setdefault("JAX_PLATFORMS", "axon,cpu")

import numpy as np

B, T, P, H, E, K, COND = 4096, 20, 20, 256, 128, 5, 5
NCORES, BC, NB = 8, 512, 4
H4 = 4 * H

_CACHE = {}


# ------------------------------------------------------------------ noise ---
def _noise():
    """Replay the reference's jax.random key tree (threefry is platform-
    deterministic). gum [T,B,20] head-major (m,y,f,fa); nrm [T,B,6] cols
    [z1, 0, zy, zf, zfa, z2]."""
    if "noise" in _CACHE:
        return _CACHE["noise"]
    import jax

    try:
        dev = jax.devices("cpu")[0]
    except RuntimeError:
        dev = jax.devices()[0]
    gum = np.zeros((T, B, 20), np.float32)
    nrm = np.zeros((T, B, 6), np.float32)
    with jax.default_device(dev):
        key = jax.random.key(42)
        for t in range(T):
            key, km, ky, kf, kfa = jax.random.split(key, 5)
            k1, k2 = jax.random.split(km)
            gum[t, :, 0:5] = np.asarray(jax.random.gumbel(k1, (B, K)))
            zm = np.asarray(jax.random.normal(k2, (B, 2)))
            nrm[t, :, 0], nrm[t, :, 5] = zm[:, 0], zm[:, 1]
            for j, kh in enumerate((ky, kf, kfa)):
                k1, k2 = jax.random.split(kh)
                gum[t, :, 5 + 5 * j:10 + 5 * j] = np.asarray(
                    jax.random.gumbel(k1, (B, K)))
                nrm[t, :, 2 + j] = np.asarray(
                    jax.random.normal(k2, (B, 1)))[:, 0]
    _CACHE["noise"] = (gum, nrm)
    return gum, nrm


# ---------------------------------------------------------------- weights ---
def _fold(inp):
    f32 = lambda k: np.asarray(inp[k], np.float32)  # noqa: E731
    We1, be1, We2, be2 = f32("We1"), f32("be1"), f32("We2"), f32("be2")
    Wk, Wr, bl = f32("Wk"), f32("Wr"), f32("bl")
    Wm, bm = f32("Wm"), f32("bm")
    Wy, by = f32("Wy"), f32("by")
    Wf, bf = f32("Wf"), f32("bf")
    Wfa, bfa = f32("Wfadj"), f32("bfadj")
    idx = np.arange(H4).reshape(4, H)
    gp = np.concatenate([idx[0], idx[1], idx[3], idx[2]])  # i,f,g,o -> i,f,o,g
    Wk_e, Wk_h, Wk_p = Wk[:E], Wk[E:E + H], Wk[E + H:]
    Wce = (We1 @ We2 @ Wk_e)[:, gp].astype(np.float32)
    Wcomb = (Wk_h + Wr)[:, gp].astype(np.float32)
    embb = be1 @ We2 + be2
    zoff = (np.cumsum(Wk_p, 0)[np.arange(T).clip(0, P - 1)] + bl
            + embb @ Wk_e)[:, gp].astype(np.float32)
    half = np.concatenate([np.full(3 * H, 0.5, np.float32),
                           np.ones(H, np.float32)])
    Wce *= half
    Wcomb *= half
    zoff *= half
    W_all = np.zeros((H, 75), np.float32)
    b_all = np.zeros((75,), np.float32)
    W_all[:, 0:5], b_all[0:5] = Wm[:, 0:5], bm[0:5]
    W_all[:, 5:10], b_all[5:10] = Wy[:, 0:5], by[0:5]
    W_all[:, 10:15], b_all[10:15] = Wf[:, 0:5], bf[0:5]
    W_all[:, 15:20], b_all[15:20] = Wfa[:, 0:5], bfa[0:5]
    W_all[:, 20:45], b_all[20:45] = Wm[:, 5:30], bm[5:30]
    W_all[:, 45:55], b_all[45:55] = Wy[:, 5:15], by[5:15]
    W_all[:, 55:65], b_all[55:65] = Wf[:, 5:15], bf[5:15]
    W_all[:, 65:75], b_all[65:75] = Wfa[:, 5:15], bfa[5:15]
    return Wce, Wcomb, zoff, W_all, b_all


def _slice_id(s, u):  # wave s slot u -> feature slice (order [i,f,o,g])
    return (s, 2 + s, 4 + s, 6 + s)[u]


# ------------------------------------------------------------ bass module ---
def _build():
    if "nc" in _CACHE:
        return _CACHE["nc"]
    import concourse.bass as bass
    import concourse.mybir as mybir
    from concourse import bacc
    from concourse.tile import TileContext
    from concourse.masks import make_identity

    F32, F16, F32R = mybir.dt.float32, mybir.dt.float16, mybir.dt.float32r
    AF, OP = mybir.ActivationFunctionType, mybir.AluOpType
    X = mybir.AxisListType.X

    def rap(src, *dims):
        """new AP reusing src's tensor/partition-dim/offset, with the free
        dims replaced by explicit (step, count) pairs (element units)."""
        return bass.AP(src.tensor, src.offset,
                       [list(src.ap[0])] + [[s_, c_] for (s_, c_) in dims])

    nc = bacc.Bacc(None, target_bir_lowering=False)
    d_sht = nc.dram_tensor("sht", [128, 2, 512], F32R, kind="ExternalInput")
    d_wcb = nc.dram_tensor("wcb", [128, 2, 8, 128], F32R, kind="ExternalInput")
    d_c0t = nc.dram_tensor("c0t", [128, 2, 512], F16, kind="ExternalInput")
    d_wce = nc.dram_tensor("wce", [6, 8 * T, 128], F32R, kind="ExternalInput")
    d_wal = nc.dram_tensor("wal", [128, 2, 75], F16, kind="ExternalInput")
    d_bal = nc.dram_tensor("bal", [75, 1], F32, kind="ExternalInput")
    d_gmb = nc.dram_tensor("gmb", [128, T, 4, 20], F32, kind="ExternalInput")
    d_nrb = nc.dram_tensor("nrb", [128, T, 4, 6], F32, kind="ExternalInput")
    d_ct0 = nc.dram_tensor("ct0", [128, 512], F32R, kind="ExternalInput")
    d_out = nc.dram_tensor("out", [4, 128, T, 75], F32, kind="ExternalOutput")

    with TileContext(nc) as tc:
        with tc.tile_pool(name="const", bufs=1) as cp, \
             tc.tile_pool(name="gw", bufs=2) as gw, \
             tc.tile_pool(name="sm", bufs=3) as sm, \
             tc.tile_pool(name="obp", bufs=3) as obp, \
             tc.tile_pool(name="ctp", bufs=2) as ctp, \
             tc.tile_pool(name="psz", bufs=4, space="PSUM") as psz, \
             tc.tile_pool(name="psp", bufs=1, space="PSUM") as psp, \
             tc.tile_pool(name="psb", bufs=2, space="PSUM") as psb, \
             tc.tile_pool(name="psc", bufs=1, space="PSUM") as psc:

            sht = cp.tile([128, 2, 512], F32R)
            nc.sync.dma_start(out=sht, in_=d_sht[:, :, :])
            wcb = cp.tile([128, 2, 8, 128], F32R)
            nc.sync.dma_start(out=wcb, in_=d_wcb[:, :, :, :])
            c0t = cp.tile([128, 2, 512], F16)
            nc.sync.dma_start(out=c0t, in_=d_c0t[:, :, :])
            wce = cp.tile([128, 8 * T, 128], F32R)
            nc.sync.dma_start(out=wce[0:6, :, :], in_=d_wce[:, :, :])
            wal = cp.tile([128, 2, 75], F16)
            nc.sync.dma_start(out=wal, in_=d_wal[:, :, :])
            bal = cp.tile([128, 1], F32)
            nc.sync.dma_start(out=bal[0:75, :], in_=d_bal[:, :])
            gmb = cp.tile([128, T, 4, 20], F32)
            nc.sync.dma_start(out=gmb, in_=d_gmb[:, :, :, :])
            nrb = cp.tile([128, T, 4, 6], F32)
            nc.sync.dma_start(out=nrb, in_=d_nrb[:, :, :, :])
            ident = cp.tile([128, 128], F32)
            make_identity(nc, ident)

            ct = ctp.tile([128, 512], F32R, tag="ct")
            nc.sync.dma_start(out=ct, in_=d_ct0[:, :])

            def preload():
                """zbase-accumulation matmuls for the next step (s=0 waves
                only: exactly fills the 4 psum slots): no cond dependency,
                emitted before the current step's sampling so the PE fills
                the sampling-tail gap."""
                lst = []
                for s in range(1):
                    for b in range(2):
                        bs = slice(256 * b, 256 * (b + 1))
                        ztA = psz.tile([128, 2, 256], F32, tag="zt")  # i,f
                        ztB = psz.tile([128, 2, 256], F32, tag="zt")  # o,g
                        for zt_, us in ((ztA, (0, 1)), (ztB, (2, 3))):
                            for u2, u in enumerate(us):
                                sl = _slice_id(s, u)
                                for k2 in range(2):
                                    nc.tensor.matmul(
                                        zt_[:, u2, :],
                                        lhsT=wcb[:, k2, sl, :],
                                        rhs=sht[:, k2, bs],
                                        start=(k2 == 0), stop=False,
                                        skip_group_check=True)
                        lst.append((ztA, ztB, s, b, bs))
                return lst

            zts = preload()
            for t in range(T):
                h2 = gw.tile([128, 2, 512], F16, tag="h2")

                def flush(pend):
                    # software-pipelined tail of a gate wave: tanh(c) and 2h
                    cs_, thB_, s_, bs_ = pend
                    tcn = gw.tile([128, 256], F16, tag="tcn")
                    nc.scalar.activation(tcn, cs_, AF.Tanh, scale=0.5)
                    nc.vector.scalar_tensor_tensor(
                        h2[:, s_, bs_], thB_[:, 0, :], 1.0, tcn,
                        OP.add, OP.mult)

                pend = None
                for ztA, ztB, s, b, bs in zts:
                    for zt_, us in ((ztA, (0, 1)), (ztB, (2, 3))):
                        for u2, u in enumerate(us):
                            nc.tensor.matmul(
                                zt_[:, u2, :],
                                lhsT=wce[0:6, 8 * t + 4 * s + u, :],
                                rhs=ct[0:6, bs],
                                start=False, stop=True,
                                skip_group_check=True)
                for s_ in range(1, 2):
                    for b_ in range(2):
                        bs_ = slice(256 * b_, 256 * (b_ + 1))
                        zA = psz.tile([128, 2, 256], F32, tag="zt")
                        zB = psz.tile([128, 2, 256], F32, tag="zt")
                        for zt_, us in ((zA, (0, 1)), (zB, (2, 3))):
                            for u2, u in enumerate(us):
                                sl = _slice_id(s_, u)
                                for k2 in range(2):
                                    nc.tensor.matmul(
                                        zt_[:, u2, :],
                                        lhsT=wcb[:, k2, sl, :],
                                        rhs=sht[:, k2, bs_],
                                        start=(k2 == 0), stop=False)
                                nc.tensor.matmul(
                                    zt_[:, u2, :],
                                    lhsT=wce[0:6, 8 * t + 4 * s_ + u, :],
                                    rhs=ct[0:6, bs_],
                                    start=False, stop=True)
                        zts.append((zA, zB, s_, b_, bs_))
                if True:
                    for ztA, ztB, s, b, bs in zts:
                        thA = gw.tile([128, 2, 256], F16, tag="thA")
                        nc.scalar.activation(thA, ztA, AF.Tanh)
                        thB = gw.tile([128, 2, 256], F16, tag="thB")
                        nc.scalar.activation(thB, ztB, AF.Tanh)
                        a1 = gw.tile([128, 256], F16, tag="a1")
                        nc.vector.scalar_tensor_tensor(
                            a1, thA[:, 1, :], 1.0, c0t[:, s, bs],
                            OP.add, OP.mult)
                        a2 = gw.tile([128, 256], F16, tag="a2")
                        nc.vector.scalar_tensor_tensor(
                            a2, thA[:, 0, :], 1.0, thB[:, 1, :],
                            OP.add, OP.mult)
                        if pend is not None:
                            flush(pend)
                        cs = gw.tile([128, 256], F16, tag="cs")
                        nc.gpsimd.tensor_add(cs, a1, a2)  # 2c
                        pend = (cs, thB, s, bs)
                flush(pend)

                # heads
                pp = psp.tile([128, 512], F32, tag="pp")
                for k2 in range(2):
                    nc.tensor.matmul(pp[0:75, :], lhsT=wal[:, k2, :],
                                     rhs=h2[:, k2, :],
                                     start=(k2 == 0), stop=(k2 == 1))
                pt = sm.tile([128, 512], F32, tag="pt")
                nc.scalar.activation(pt[0:75, :], pp[0:75, :], AF.Identity,
                                     bias=bal[0:75, :])
                pb = psb.tile([128, 4, 75], F32, tag="pb")
                for j in range(4):
                    nc.tensor.transpose(pb[:, j, :],
                                        pt[0:75, 128 * j:128 * (j + 1)],
                                        ident[0:75, 0:75])

                if t + 1 < T:
                    zts = preload()

                # output staging with transforms (ob col layout:
                # [aM mu_lo s_lo mu_la s_la rho | aY muY sY | aF muF sF |
                #  aFA muFA sFA])
                ob = obp.tile([128, 4, 75], F32, tag="ob")
                nc.vector.tensor_copy(          # mu_lo, mu_la
                    rap(ob[:, :, 5:6], (75, 4), (10, 2), (1, 5)),
                    rap(pb[:, :, 20:21], (75, 4), (10, 2), (1, 5)))
                nc.scalar.activation(          # s_lo, s_la
                    rap(ob[:, :, 10:11], (75, 4), (10, 2), (1, 5)),
                    rap(pb[:, :, 25:26], (75, 4), (10, 2), (1, 5)), AF.Exp)
                nc.scalar.activation(ob[:, :, 25:30], pb[:, :, 40:45],
                                     AF.Tanh)  # rho
                nc.vector.tensor_copy(          # mu y/f/fa
                    rap(ob[:, :, 35:36], (75, 4), (15, 3), (1, 5)),
                    rap(pb[:, :, 45:46], (75, 4), (10, 3), (1, 5)))
                nc.scalar.activation(          # sig y/f/fa
                    rap(ob[:, :, 40:41], (75, 4), (15, 3), (1, 5)),
                    rap(pb[:, :, 50:51], (75, 4), (10, 3), (1, 5)), AF.Exp)
                e_ = sm.tile([128, 4, 20], F32, tag="e_")
                nc.scalar.activation(e_, pb[:, :, 0:20], AF.Exp)
                ssum = sm.tile([128, 4, 4], F32, tag="ssum")
                nc.vector.tensor_reduce(
                    ssum, e_.rearrange("p t (a b) -> p t a b", a=4), X, OP.add)
                rcp = sm.tile([128, 4, 4], F32, tag="rcp")
                nc.vector.reciprocal(rcp, ssum)
                nc.gpsimd.tensor_tensor(   # alphas merge
                    rap(ob[:, :, 0:1], (75, 4), (1, 5)),
                    e_[:, :, 0:5],
                    rap(rcp[:, :, 0:1], (4, 4), (0, 5)), op=OP.mult)
                nc.gpsimd.tensor_tensor(   # alphas y/f/fa
                    rap(ob[:, :, 30:31], (75, 4), (15, 3), (1, 5)),
                    e_[:, :, 5:20].rearrange("p t (a b) -> p t a b", a=3),
                    rap(rcp[:, :, 1:2], (4, 4), (1, 3), (0, 5)), op=OP.mult)

                # selection: onehot(argmax(logits+gumbel))
                q = sm.tile([128, 4, 20], F32, tag="q")
                nc.vector.tensor_add(q, pb[:, :, 0:20], gmb[:, t, :, :])
                qmax = sm.tile([128, 4, 4], F32, tag="qmax")
                nc.vector.tensor_reduce(
                    qmax, q.rearrange("p t (a b) -> p t a b", a=4), X, OP.max)
                oh = sm.tile([128, 4, 20], F32, tag="oh")
                nc.vector.tensor_tensor(
                    oh.rearrange("p t (a b) -> p t a b", a=4),
                    q.rearrange("p t (a b) -> p t a b", a=4),
                    rap(qmax[:, :, 0:1], (4, 4), (1, 4), (0, 5)),
                    op=OP.is_equal)
                # masked params (sigmas already exp'd in ob)
                # mm: [mu_lo s_lo mu_la s_la | rho] x k (ob natural order)
                mm = sm.tile([128, 4, 25], F32, tag="mm")
                nc.vector.tensor_tensor(
                    mm[:, :, 0:20].rearrange("p t (a b) -> p t a b", a=4),
                    ob[:, :, 5:25].rearrange("p t (a b) -> p t a b", a=4),
                    rap(oh[:, :, 0:1], (20, 4), (0, 4), (1, 5)), op=OP.mult)
                nc.vector.tensor_tensor(mm[:, :, 20:25], ob[:, :, 25:30],
                                        oh[:, :, 0:5], op=OP.mult)
                # mo: [muY sY muF sF muFA sFA] x k
                mo = sm.tile([128, 4, 30], F32, tag="mo")
                nc.vector.tensor_tensor(   # mus
                    rap(mo[:, :, 0:1], (30, 4), (10, 3), (1, 5)),
                    rap(ob[:, :, 35:36], (75, 4), (15, 3), (1, 5)),
                    rap(oh[:, :, 5:6], (20, 4), (5, 3), (1, 5)), op=OP.mult)
                nc.vector.tensor_tensor(   # sigmas
                    rap(mo[:, :, 5:6], (30, 4), (10, 3), (1, 5)),
                    rap(ob[:, :, 40:41], (75, 4), (15, 3), (1, 5)),
                    rap(oh[:, :, 5:6], (20, 4), (5, 3), (1, 5)), op=OP.mult)
                # sel: [mu_lo s_lo mu_la s_la rho muY sY muF sF muFA sFA]
                sel = sm.tile([128, 4, 12], F32, tag="sel")
                nc.vector.tensor_reduce(
                    sel[:, :, 0:4],
                    mm[:, :, 0:20].rearrange("p t (a b) -> p t a b", a=4),
                    X, OP.add)
                nc.vector.tensor_reduce(sel[:, :, 4:5], mm[:, :, 20:25],
                                        X, OP.add)
                nc.vector.tensor_reduce(
                    sel[:, :, 5:11],
                    mo.rearrange("p t (a b) -> p t a b", a=6), X, OP.add)
                # samples -> next cond (layout B)
                r2 = sm.tile([128, 4, 1], F32, tag="r2")
                nc.vector.tensor_mul(r2, sel[:, :, 4:5], sel[:, :, 4:5])
                sq = sm.tile([128, 4, 1], F32, tag="sq")
                nc.vector.tensor_scalar(sq, r2, -0.5, 1.0, OP.mult, OP.add)
                u1 = sm.tile([128, 4, 1], F32, tag="u1")
                nc.vector.tensor_mul(u1, sel[:, :, 4:5], nrb[:, t, :, 0:1])
                u2 = sm.tile([128, 4, 1], F32, tag="u2")
                nc.vector.tensor_mul(u2, sq, nrb[:, t, :, 5:6])
                mix = sm.tile([128, 4, 1], F32, tag="mix")
                nc.vector.tensor_add(mix, u1, u2)
                cb = sm.tile([128, 4, 8], F32, tag="cb")
                nc.vector.memset(cb[:, :, 5:6], 1.0)
                tm = sm.tile([128, 4, 2], F32, tag="tm")
                nc.vector.tensor_mul(tm, rap(sel[:, :, 1:2], (12, 4), (2, 2)),
                                     nrb[:, t, :, 0:2])
                nc.vector.tensor_add(cb[:, :, 0:2],
                                     rap(sel[:, :, 0:1], (12, 4), (2, 2)), tm)
                w_ = sm.tile([128, 4, 1], F32, tag="w_")
                nc.vector.tensor_mul(w_, sel[:, :, 3:4], mix)
                nc.vector.tensor_add(cb[:, :, 1:2], sel[:, :, 2:3], w_)
                to = sm.tile([128, 4, 3], F32, tag="to")
                nc.vector.tensor_mul(to, rap(sel[:, :, 6:7], (12, 4), (2, 3)),
                                     nrb[:, t, :, 2:5])
                nc.vector.tensor_add(cb[:, :, 2:5],
                                     rap(sel[:, :, 5:6], (12, 4), (2, 3)), to)

                if t + 1 < T:
                    cps = psc.tile([128, 512], F32, tag="cps")
                    for j in range(4):
                        nc.tensor.transpose(
                            cps[0:6, 128 * j:128 * (j + 1)],
                            cb[:, j, 0:6], ident)
                    ct = ctp.tile([128, 512], F32R, tag="ct")
                    nc.scalar.activation(ct[0:6, :], cps[0:6, :], AF.Copy)

                oap = d_out[:, :, t, :]
                oap2 = bass.AP(oap.tensor, oap.offset,
                               [oap.ap[1], oap.ap[0], oap.ap[2]])
                nc.sync.dma_start(out=oap2, in_=ob)

    nc.finalize()
    _CACHE["nc"] = nc
    return nc


# ----------------------------------------------------------------- kernel ---
def kernel(**inputs):
    from concourse.bass_utils import run_bass_kernel_spmd

    assert int(inputs["steps_n"]) == T
    conditions = np.asarray(inputs["conditions"], np.float32)
    state_h = np.asarray(inputs["state_h"], np.float32)
    state_c = np.asarray(inputs["state_c"], np.float32)
    Wce, Wcomb, zoff, W_all, b_all = _fold(inputs)
    gum, nrm = _noise()
    nc = _build()

    wcb_h = np.ascontiguousarray(
        Wcomb.reshape(2, 128, 8, 128).transpose(1, 0, 2, 3))  # [128,2,8,128]
    wce_h = np.zeros((6, 8 * T, 128), np.float32)
    for t in range(T):
        for s in range(2):
            for u in range(4):
                sl = _slice_id(s, u)
                blk = 8 * t + 4 * s + u
                wce_h[0:5, blk, :] = Wce[:, 128 * sl:128 * (sl + 1)]
                wce_h[5, blk, :] = zoff[t, 128 * sl:128 * (sl + 1)]
    wal_h = np.ascontiguousarray(
        (0.5 * W_all).astype(np.float16).reshape(2, 128, 75).transpose(
            1, 0, 2))                                      # [128, 2, 75]
    bal_h = np.ascontiguousarray(b_all.reshape(75, 1))

    in_maps = []
    for c in range(NCORES):
        b0 = c * BC
        sht = np.ascontiguousarray(
            state_h[b0:b0 + BC].T.reshape(2, 128, BC).transpose(1, 0, 2))
        c0t = np.ascontiguousarray(
            state_c[b0:b0 + BC].T.reshape(2, 128, BC).transpose(
                1, 0, 2)).astype(np.float16)
        gmb = np.ascontiguousarray(
            gum[:, b0:b0 + BC, :].reshape(T, NB, 128, 20).transpose(
                2, 0, 1, 3))
        nrb = np.ascontiguousarray(
            nrm[:, b0:b0 + BC, :].reshape(T, NB, 128, 6).transpose(
                2, 0, 1, 3))
        ct0 = np.zeros((128, 512), np.float32)
        ct0[0:5, :] = conditions[b0:b0 + BC, 0, :].T
        ct0[5, :] = 1.0
        in_maps.append(dict(
            sht=sht, wcb=wcb_h, c0t=c0t, wce=wce_h,
            wal=wal_h, bal=bal_h,
            gmb=gmb, nrb=nrb, ct0=ct0))

    res = run_bass_kernel_spmd(nc, in_maps, core_ids=list(range(NCORES)))
    _CACHE["last_result"] = res
    full = np.concatenate(
        [np.asarray(res.results[c]["out"]).reshape(BC, T, 75)
         for c in range(NCORES)], 0)
    return (np.ascontiguousarray(full[:, :, 0:30]),
            np.ascontiguousarray(full[:, :, 30:45]),
            np.ascontiguousarray(full[:, :, 45:60]),
            np.ascontiguousarray(full[:, :, 60:75]))
